# revision 5
# baseline (speedup 1.0000x reference)
"""K-means quantizer (nearest-centroid assignment) for Trainium2, 8-core SPMD.

Problem: feat [16, 4096, 768] f32, clusters [1024, 768] f32
         -> indices [16, 4096] int32 = argmin_c ||feat - clusters[c]||^2

Math: argmin_c d2 = argmax_c (feat . c - 0.5*||c||^2)   (per-token ||x||^2 dropped)

Device kernel (per core, data-parallel over batch: 2 batches = 8192 tokens/core):
  - GEMM s[t, c] = x.c + bias via 3-term bf16 split (xhi*chi + xhi*clo + xlo*chi),
    each term a bf16 matmul at full PE rate; abs err ~1e-4 (fp32-grade for argmin).
    The -0.5*||c||^2 bias rides in as a K=3 matmul of bf16 bias rows vs ones.
  - argmax over c per token via DVE max (top-8) + max_index.
  - indices accumulated in SBUF, PE-transposed at the end so the final DMA writes
    contiguous 256B runs.

Host side: transposes + hi/lo bf16 splits of feat and clusters (kernel() input prep).
"""

import numpy as np
import ml_dtypes
from contextlib import ExitStack

import concourse.bass as bass
import concourse.bacc as bacc
import concourse.tile as tile
import concourse.mybir as mybir
from concourse._compat import with_exitstack
from concourse.bass_utils import run_bass_kernel_spmd

B, T_SEQ, D = 16, 4096, 768
C = 1024
N_CORES = 8
T_CORE = (B // N_CORES) * T_SEQ          # 8192 tokens per core
KCH = D // 128                            # 6 contraction chunks
T_TILE = 512                              # tokens per DMA tile
N_TILES = T_CORE // T_TILE                # 16
M_SUB = T_TILE // 128                     # 4 matmul subtiles per DMA tile

BF16 = mybir.dt.bfloat16
F32 = mybir.dt.float32
U16 = mybir.dt.uint16


@with_exitstack
def kmeans_device_kernel(ctx: ExitStack, tc: tile.TileContext,
                         fhi, flo, chi, clo, bias3, ones3, ident, out):
    nc = tc.nc

    const_pool = ctx.enter_context(tc.tile_pool(name="const", bufs=1))
    feat_pool = ctx.enter_context(tc.tile_pool(name="feat", bufs=3))
    s_pool = ctx.enter_context(tc.tile_pool(name="s", bufs=3))
    small_pool = ctx.enter_context(tc.tile_pool(name="small", bufs=2))
    acc_pool = ctx.enter_context(tc.tile_pool(name="acc", bufs=1))
    psum_pool = ctx.enter_context(tc.tile_pool(name="psum", bufs=3, space="PSUM"))
    psum_idx_pool = ctx.enter_context(tc.tile_pool(name="psum_idx", bufs=1, space="PSUM"))

    # ---- constants ----
    chi_sb = const_pool.tile([128, KCH, C], BF16)
    nc.sync.dma_start(chi_sb[:], chi.rearrange("(k p) c -> p k c", p=128))
    clo_sb = const_pool.tile([128, KCH, C], BF16)
    nc.sync.dma_start(clo_sb[:], clo.rearrange("(k p) c -> p k c", p=128))
    bias_sb = const_pool.tile([3, C], BF16)
    nc.sync.dma_start(bias_sb[:], bias3)
    ones_sb = const_pool.tile([3, 128], BF16)
    nc.sync.dma_start(ones_sb[:], ones3)
    ident_sb = const_pool.tile([128, 128], F32)
    nc.sync.dma_start(ident_sb[:], ident)

    # ---- index accumulator: [128, 64 subtiles, 8] ----
    n_sub = N_TILES * M_SUB
    idx_acc = acc_pool.tile([128, n_sub, 8], U16)

    fhi_r = fhi.rearrange("(k p) t -> p k t", p=128)
    flo_r = flo.rearrange("(k p) t -> p k t", p=128)

    for i in range(N_TILES):
        fhi_sb = feat_pool.tile([128, KCH, T_TILE], BF16, tag="fhi")
        nc.sync.dma_start(fhi_sb[:], fhi_r[:, :, i * T_TILE:(i + 1) * T_TILE])
        flo_sb = feat_pool.tile([128, KCH, T_TILE], BF16, tag="flo")
        nc.sync.dma_start(flo_sb[:], flo_r[:, :, i * T_TILE:(i + 1) * T_TILE])

        for m in range(M_SUB):
            sub = i * M_SUB + m
            ts = slice(m * 128, (m + 1) * 128)

            s_psum = psum_pool.tile([128, C], F32, tag="s_psum")
            for n in range(0, C, 512):
                ns = slice(n, n + 512)
                nc.tensor.matmul(s_psum[:, ns], ones_sb[:, :], bias_sb[:, ns],
                                 start=True, stop=False)
                for k in range(KCH):
                    nc.tensor.matmul(s_psum[:, ns], fhi_sb[:, k, ts], chi_sb[:, k, ns],
                                     start=False, stop=False)
                    nc.tensor.matmul(s_psum[:, ns], fhi_sb[:, k, ts], clo_sb[:, k, ns],
                                     start=False, stop=False)
                    nc.tensor.matmul(s_psum[:, ns], flo_sb[:, k, ts], chi_sb[:, k, ns],
                                     start=False, stop=(k == KCH - 1))

            s_sb = s_pool.tile([128, C], F32, tag="s_sb")
            nc.scalar.copy(s_sb[:], s_psum[:])

            max8 = small_pool.tile([128, 8], F32, tag="max8")
            nc.vector.max(max8[:], s_sb[:])
            nc.vector.max_index(idx_acc[:, sub, :], max8[:], s_sb[:])

    # ---- emit indices: uint16 [128, 64] -> f32 -> PE transpose -> [64, 128] -> DRAM ----
    # (f32 is exact for 0..1023; the verifier rejects integer Ldweights)
    idxT = acc_pool.tile([128, n_sub], F32)
    nc.vector.tensor_copy(idxT[:], idx_acc[:, :, 0])
    idx_ps = psum_idx_pool.tile([n_sub, 128], F32)
    nc.tensor.transpose(idx_ps[:], idxT[:], ident_sb[:])
    idx_out = acc_pool.tile([n_sub, 128], F32)
    nc.vector.tensor_copy(idx_out[:], idx_ps[:])
    nc.sync.dma_start(out.rearrange("(j p) -> j p", p=128), idx_out[:])


_CACHED_NC = None


def _build():
    global _CACHED_NC
    if _CACHED_NC is not None:
        return _CACHED_NC
    nc = bacc.Bacc("TRN2", target_bir_lowering=False, debug=False,
                   enable_asserts=False, num_devices=N_CORES)
    fhi = nc.dram_tensor("fhi", [D, T_CORE], BF16, kind="ExternalInput").ap()
    flo = nc.dram_tensor("flo", [D, T_CORE], BF16, kind="ExternalInput").ap()
    chi = nc.dram_tensor("chi", [D, C], BF16, kind="ExternalInput").ap()
    clo = nc.dram_tensor("clo", [D, C], BF16, kind="ExternalInput").ap()
    bias3 = nc.dram_tensor("bias3", [3, C], BF16, kind="ExternalInput").ap()
    ones3 = nc.dram_tensor("ones3", [3, 128], BF16, kind="ExternalInput").ap()
    ident = nc.dram_tensor("ident", [128, 128], F32, kind="ExternalInput").ap()
    out = nc.dram_tensor("out", [T_CORE], F32, kind="ExternalOutput").ap()
    with tile.TileContext(nc) as tc:
        kmeans_device_kernel(tc, fhi, flo, chi, clo, bias3, ones3, ident, out)
    nc.compile()
    _CACHED_NC = nc
    return nc


def _split_bf16(a):
    hi = a.astype(ml_dtypes.bfloat16)
    lo = (a - hi.astype(np.float32)).astype(ml_dtypes.bfloat16)
    return hi, lo


def _prep_inputs(feat, clusters):
    feat = np.asarray(feat, dtype=np.float32)
    clusters = np.asarray(clusters, dtype=np.float32)

    cT = np.ascontiguousarray(clusters.T)                      # [768, 1024]
    chi, clo = _split_bf16(cT)

    c2 = np.einsum("cd,cd->c", clusters.astype(np.float64), clusters.astype(np.float64))
    bias = (-0.5 * c2)[None, :].astype(np.float32)             # [1, 1024]
    b0 = bias.astype(ml_dtypes.bfloat16)
    r = bias - b0.astype(np.float32)
    b1 = r.astype(ml_dtypes.bfloat16)
    b2 = (r - b1.astype(np.float32)).astype(ml_dtypes.bfloat16)
    bias3 = np.ascontiguousarray(np.concatenate([b0, b1, b2], 0))
    ones3 = np.ones((3, 128), ml_dtypes.bfloat16)
    ident = np.eye(128, dtype=np.float32)

    shared = {"chi": chi, "clo": clo, "bias3": bias3, "ones3": ones3, "ident": ident}

    in_maps = []
    bpc = B // N_CORES
    for j in range(N_CORES):
        x = feat[j * bpc:(j + 1) * bpc].reshape(T_CORE, D)
        xT = np.ascontiguousarray(x.T)                         # [768, 8192]
        fhi, flo = _split_bf16(xT)
        in_maps.append({"fhi": fhi, "flo": flo, **shared})
    return in_maps


def kernel(feat, clusters, _trace=False):
    nc = _build()
    in_maps = _prep_inputs(feat, clusters)
    res = run_bass_kernel_spmd(nc, in_maps, list(range(N_CORES)), trace=_trace)
    out = np.concatenate([res.results[j]["out"] for j in range(N_CORES)])
    result = out.reshape(B, T_SEQ).astype(np.int32)
    if _trace:
        return result, res
    return result


# revision 14
# speedup vs baseline: 11823.1523x; 11823.1523x over previous
"""K-means quantizer (nearest-centroid assignment) for Trainium2, 8-core SPMD.

Problem: feat [16, 4096, 768] f32, clusters [1024, 768] f32
         -> indices [16, 4096] int32 = argmin_c ||feat - clusters[c]||^2

Math: argmin_c d2 = argmax_c (feat . c - 0.5*||c||^2)   (per-token ||x||^2 dropped)

Device kernel (per core, data-parallel over batch: 2 batches = 8192 tokens/core):
  - GEMM s[t, c] = x.c + bias via 3-term bf16 split (xhi*chi + xhi*clo + xlo*chi),
    each term a bf16 matmul at full PE rate; abs err ~1e-4 (fp32-grade for argmin).
    The -0.5*||c||^2 bias rides in as a K=3 matmul of bf16 bias rows vs ones.
  - argmax over c per token via DVE max (top-8) + max_index.
  - indices accumulated in SBUF, PE-transposed at the end so the final DMA writes
    contiguous 256B runs.

Host side: transposes + hi/lo bf16 splits of feat and clusters (kernel() input prep).
"""

import numpy as np
import ml_dtypes
from contextlib import ExitStack

import concourse.bass as bass
import concourse.bacc as bacc
import concourse.tile as tile
import concourse.mybir as mybir
from concourse._compat import with_exitstack
from concourse.bass_utils import run_bass_kernel_spmd

B, T_SEQ, D = 16, 4096, 768
C = 1024
N_CORES = 8
T_CORE = (B // N_CORES) * T_SEQ          # 8192 tokens per core
KCH = D // 128                            # 6 contraction chunks
T_TILE = 512                              # tokens per DMA tile
N_TILES = T_CORE // T_TILE                # 16
M_SUB = T_TILE // 128                     # 4 matmul subtiles per DMA tile

BF16 = mybir.dt.bfloat16
F32 = mybir.dt.float32
U16 = mybir.dt.uint16


@with_exitstack
def kmeans_device_kernel(ctx: ExitStack, tc: tile.TileContext,
                         fhi, flo, chi, clo, bias3, ident, out):
    nc = tc.nc

    const_pool = ctx.enter_context(tc.tile_pool(name="const", bufs=1))
    feat_pool = ctx.enter_context(tc.tile_pool(name="feat", bufs=3))
    s_pool = ctx.enter_context(tc.tile_pool(name="s", bufs=3))
    small_pool = ctx.enter_context(tc.tile_pool(name="small", bufs=2))
    acc_pool = ctx.enter_context(tc.tile_pool(name="acc", bufs=1))
    psum_pool = ctx.enter_context(tc.tile_pool(name="psum", bufs=3, space="PSUM"))
    psum_idx_pool = ctx.enter_context(tc.tile_pool(name="psum_idx", bufs=1, space="PSUM"))

    # ---- constants ----
    chi_sb = const_pool.tile([128, KCH, C], BF16)
    nc.sync.dma_start(chi_sb[:], chi.rearrange("(k p) c -> p k c", p=128))
    clo_sb = const_pool.tile([128, KCH, C], BF16)
    nc.sync.dma_start(clo_sb[:], clo.rearrange("(k p) c -> p k c", p=128))
    bias_sb = const_pool.tile([128, C], F32)
    nc.sync.dma_start(bias_sb[:], bias3)
    ident_sb = const_pool.tile([128, 128], F32)
    nc.sync.dma_start(ident_sb[:], ident)

    # ---- index accumulator: [128, 64 subtiles, 8] ----
    n_sub = N_TILES * M_SUB
    idx_acc = acc_pool.tile([128, n_sub, 8], U16)

    fhi_r = fhi.rearrange("(k p) t -> p k t", p=128)
    flo_r = flo.rearrange("(k p) t -> p k t", p=128)

    for i in range(N_TILES):
        fhi_sb = feat_pool.tile([128, KCH, T_TILE], BF16, tag="fhi")
        nc.sync.dma_start(fhi_sb[:], fhi_r[:, :, i * T_TILE:(i + 1) * T_TILE])
        flo_sb = feat_pool.tile([128, KCH, T_TILE], BF16, tag="flo")
        nc.sync.dma_start(flo_sb[:], flo_r[:, :, i * T_TILE:(i + 1) * T_TILE])

        for m in range(M_SUB):
            sub = i * M_SUB + m
            ts = slice(m * 128, (m + 1) * 128)

            s_psum = psum_pool.tile([128, C], F32, tag="s_psum")
            for n in range(0, C, 512):
                ns = slice(n, n + 512)
                for k in range(KCH):
                    nc.tensor.matmul(s_psum[:, ns], fhi_sb[:, k, ts], chi_sb[:, k, ns],
                                     start=(k == 0), stop=False)
                    nc.tensor.matmul(s_psum[:, ns], fhi_sb[:, k, ts], clo_sb[:, k, ns],
                                     start=False, stop=False)
                    nc.tensor.matmul(s_psum[:, ns], flo_sb[:, k, ts], chi_sb[:, k, ns],
                                     start=False, stop=(k == KCH - 1))

            # drain PSUM -> SBUF fused with the exact-f32 bias add (DVE)
            s_sb = s_pool.tile([128, C], F32, tag="s_sb")
            nc.vector.tensor_add(s_sb[:], s_psum[:], bias_sb[:])

            max8 = small_pool.tile([128, 8], F32, tag="max8")
            nc.vector.max(max8[:], s_sb[:])
            nc.vector.max_index(idx_acc[:, sub, :], max8[:], s_sb[:])

    # ---- emit indices: uint16 [128, 64] -> f32 -> PE transpose -> [64, 128] -> DRAM ----
    # (f32 is exact for 0..1023; the verifier rejects integer Ldweights)
    idxT = acc_pool.tile([128, n_sub], F32)
    nc.vector.tensor_copy(idxT[:], idx_acc[:, :, 0])
    idx_ps = psum_idx_pool.tile([n_sub, 128], F32)
    nc.tensor.transpose(idx_ps[:], idxT[:], ident_sb[:])
    idx_out = acc_pool.tile([n_sub, 128], F32)
    nc.vector.tensor_copy(idx_out[:], idx_ps[:])
    nc.sync.dma_start(out.rearrange("(j p) -> j p", p=128), idx_out[:])


_CACHED_NC = None


def _build():
    global _CACHED_NC
    if _CACHED_NC is not None:
        return _CACHED_NC
    nc = bacc.Bacc("TRN2", target_bir_lowering=False, debug=False,
                   enable_asserts=False, num_devices=N_CORES)
    fhi = nc.dram_tensor("fhi", [D, T_CORE], BF16, kind="ExternalInput").ap()
    flo = nc.dram_tensor("flo", [D, T_CORE], BF16, kind="ExternalInput").ap()
    chi = nc.dram_tensor("chi", [D, C], BF16, kind="ExternalInput").ap()
    clo = nc.dram_tensor("clo", [D, C], BF16, kind="ExternalInput").ap()
    bias3 = nc.dram_tensor("bias3", [128, C], F32, kind="ExternalInput").ap()
    ident = nc.dram_tensor("ident", [128, 128], F32, kind="ExternalInput").ap()
    out = nc.dram_tensor("out", [T_CORE], F32, kind="ExternalOutput").ap()
    with tile.TileContext(nc) as tc:
        kmeans_device_kernel(tc, fhi, flo, chi, clo, bias3, ident, out)
    nc.compile()
    _CACHED_NC = nc
    return nc


def _split_bf16(a):
    hi = a.astype(ml_dtypes.bfloat16)
    lo = (a - hi.astype(np.float32)).astype(ml_dtypes.bfloat16)
    return hi, lo


def _prep_inputs(feat, clusters):
    feat = np.asarray(feat, dtype=np.float32)
    clusters = np.asarray(clusters, dtype=np.float32)

    cT = np.ascontiguousarray(clusters.T)                      # [768, 1024]
    chi, clo = _split_bf16(cT)

    c2 = np.einsum("cd,cd->c", clusters.astype(np.float64), clusters.astype(np.float64))
    bias_row = (-0.5 * c2)[None, :].astype(np.float32)
    bias3 = np.ascontiguousarray(np.broadcast_to(bias_row, (128, C)))      # [128, 1024]
    ident = np.eye(128, dtype=np.float32)

    shared = {"chi": chi, "clo": clo, "bias3": bias3, "ident": ident}

    in_maps = []
    bpc = B // N_CORES
    for j in range(N_CORES):
        x = feat[j * bpc:(j + 1) * bpc].reshape(T_CORE, D)
        xT = np.ascontiguousarray(x.T)                         # [768, 8192]
        fhi, flo = _split_bf16(xT)
        in_maps.append({"fhi": fhi, "flo": flo, **shared})
    return in_maps


def kernel(feat, clusters, _trace=False):
    nc = _build()
    in_maps = _prep_inputs(feat, clusters)
    res = run_bass_kernel_spmd(nc, in_maps, list(range(N_CORES)), trace=_trace)
    out = np.concatenate([res.results[j]["out"] for j in range(N_CORES)])
    result = out.reshape(B, T_SEQ).astype(np.int32)
    if _trace:
        return result, res
    return result
